# revision 40
# baseline (speedup 1.0000x reference)
"""Forward-kinematics (SMPL 24-joint) Bass kernel for 8 trn2 NeuronCores.

Layout: batch sharded 8 ways (data parallel). Per core, chunks of
[128 partitions x NF] batch elements; each partition row holds NF
elements' data contiguously (element-major: 216 floats per element for
rotations), so every DMA is a dense 2D copy.

Compute: joints processed in 9 tree-level groups with uniform child /
parent index strides, so one multi-dim-AP instruction covers a whole
group. Rotation update G[j] = G[parent] @ L[j] is done as 9 broadcast
muls (one per (i,k)) into per-k product tiles + 2 adds, writing G[j]
in-place over L[j] in the input tile. Position update
p[j] = p[parent] + G[parent] @ offsets[j] runs off the critical path:
per-joint muls on ScalarE (offsets replicated host-side into a [128, 72]
per-partition-scalar tile) and grouped adds on GPSIMD, while the serial
rotation chain stays entirely on VectorE (cross-engine handoffs inside
the chain measured slower every time). Broadcast (0-step) operands are
always passed as in1 — on in0 they cost ~30% extra on the DVE.
"""

import contextlib
import sys

sys.path.insert(0, "/opt/trn_rl_repo")

import numpy as np

import concourse.bass as bass
import concourse.mybir as mybir
import concourse.tile as tile
from concourse.bass_utils import run_bass_kernel_spmd

F32 = mybir.dt.float32

NUM_JOINTS = 24
PARENTS = [-1, 0, 0, 0, 1, 2, 3, 4, 5, 6, 7, 8, 9, 9, 9, 12, 13, 14, 16, 17, 18, 19, 20, 21]
B = 262144
NCORES = 8
BCORE = B // NCORES  # 32768
P = 128

# (j0, group_size, parent0, parent_step) — topological groups where both the
# child joints and their parents are index-arithmetic (parent_step 0 = shared
# parent broadcast across the group).
GROUPS = [
    (1, 3, 0, 0),
    (4, 3, 1, 1),
    (7, 3, 4, 1),
    (10, 3, 7, 1),
    (13, 2, 9, 0),
    (15, 3, 12, 1),
    (18, 2, 16, 1),
    (20, 2, 18, 1),
    (22, 2, 20, 1),
]

# Engine assignment knobs (flip for load-balancing experiments).
MUL_K_ENGINES = ["vector", "vector", "vector"]  # engine for k=0,1,2 rot muls
ADD_ENGINES = ["vector", "vector"]              # engines for the two rot adds
POS_MODE = "act_pool"      # "stt_dve" (fused STT chains on DVE) or "act_pool"
                           # (muls on ScalarE, grouped adds on GPSIMD)
NF = 64                    # free-dim batch elements per chunk

LAST_RESULT = None         # test.py reads exec_time_ns off this


def _eng(nc, name):
    return {"vector": nc.vector, "gpsimd": nc.gpsimd, "scalar": nc.scalar}[name]


_TPB_ENGINES = ("EngineType.DVE", "EngineType.Pool", "EngineType.Activation",
                "EngineType.PE")


def _split_waits(nc):
    """Walrus can embed only ONE sync-wait per TPB compute instruction and
    refuses to compile more. Hoist extra waits onto same-engine NoOps inserted
    immediately before the overloaded instruction (identical semantics: the
    engine stream stalls at the same point)."""
    uid = 0
    for blk in nc.m.functions[0].blocks:
        il = blk.instructions
        out, changed = [], False
        for ins in il:
            si = ins.sync_info
            waits = (si.on_wait if si else None) or []
            if len(waits) > 1:
                for w in waits[:-1]:
                    nop = mybir.InstNoOp(name=f"waitnop-{uid}")
                    uid += 1
                    nop.engine = ins.engine
                    nop.sync_info = mybir.SyncInfo(on_wait=[w], on_update=[])
                    out.append(nop)
                si.on_wait = [waits[-1]]
                changed = True
            out.append(ins)
        if changed:
            blk.instructions = out


def build(bcore=BCORE, nf=NF, mul_k_engines=None, add_engines=None, pos_mode=None,
          repeat=1, split_waits=True, do_dma=True, do_compute=True, io_bufs=2,
          pos_add_engine="gpsimd"):
    mul_k_engines = mul_k_engines or MUL_K_ENGINES
    add_engines = add_engines or ADD_ENGINES
    pos_mode = pos_mode or POS_MODE
    nchunk = bcore // (P * nf)
    assert nchunk * P * nf == bcore

    nc = bass.Bass()
    lr = nc.dram_tensor("local_rots", [bcore, 216], F32, kind="ExternalInput")
    rp = nc.dram_tensor("root_pos", [bcore, 3], F32, kind="ExternalInput")
    ob = nc.dram_tensor("offsets_bcast", [P, 72], F32, kind="ExternalInput")
    g_out = nc.dram_tensor("grots", [bcore, 216], F32, kind="ExternalOutput")
    p_out = nc.dram_tensor("gpos", [bcore, 72], F32, kind="ExternalOutput")

    lr_v = lr[:].rearrange("(c p n) e -> c p (n e)", p=P, n=nf)
    rp_v = rp[:].rearrange("(c p n) e -> c p (n e)", p=P, n=nf)
    go_v = g_out[:].rearrange("(c p n) e -> c p (n e)", p=P, n=nf)
    po_v = p_out[:].rearrange("(c p n) e -> c p (n e)", p=P, n=nf)

    mult = mybir.AluOpType.mult
    add = mybir.AluOpType.add

    with tile.TileContext(nc) as tc:
        with (
            tc.tile_pool(name="io", bufs=io_bufs) as io_pool,
            tc.tile_pool(name="tmp", bufs=1) as tmp_pool,
            tc.tile_pool(name="const", bufs=1) as const_pool,
        ):
            o_tile = const_pool.tile([P, 72], F32)
            nc.sync.dma_start(o_tile[:], ob[:])
            o_col = lambda j, k: o_tile[:, j * 3 + k : j * 3 + k + 1]

            rep_ctx = (
                tc.For_i(
                    0, repeat, 1,
                    hint_engines=(
                        mybir.EngineType.DVE,
                        mybir.EngineType.Pool,
                        mybir.EngineType.Activation,
                        mybir.EngineType.SP,
                    ),
                )
                if repeat > 1
                else contextlib.nullcontext()
            )
            with rep_ctx:
                tiles = {}

                def stage_in(c):
                    # Write-absorber: claim the recycled SBUF slot with a tiny
                    # DVE write BEFORE the DMA fills it, so the slot's
                    # write-after-read waits (vs the chunk-2-ago output DMA)
                    # land here instead of on a real compute instruction.
                    g_tile = io_pool.tile([P, nf * 216], F32, tag="io_rots",
                                          name="g_tile")
                    pin_tile = io_pool.tile([P, nf * 3], F32, tag="io_rootpos",
                                            name="pin_tile")
                    pos_tile = io_pool.tile([P, nf * 72], F32, tag="io_pos",
                                            name="pos_tile")
                    nc.vector.memset(g_tile[:, 0:1], 0.0)
                    if do_dma:
                        nc.sync.dma_start(g_tile[:], lr_v[c])
                        nc.sync.dma_start(pin_tile[:], rp_v[c])
                    else:
                        nc.vector.memset(pin_tile[:, 0:1], 0.0)
                    tiles[c] = (g_tile, pin_tile, pos_tile)

                stage_in(0)
                for c in range(nchunk):
                    g_tile, pin_tile, pos_tile = tiles.pop(c)

                    # 5-D view of the rotation data: [p, n, j, i, k]
                    g5 = g_tile.rearrange("p (n j i k) -> p n j i k", n=nf, j=24, i=3, k=3)
                    g3 = g_tile.rearrange("p (n e) -> p n e", n=nf)
                    pos3 = pos_tile.rearrange("p (n j i) -> p n j i", n=nf, j=24)

                    # Wait-absorbers: TPB compute instructions can embed only
                    # ONE sync wait; walrus refuses to compile an instruction
                    # that needs more. Touch every freshly-DMA'd / released
                    # region with tiny same-engine ops first so each carries
                    # exactly one wait and the real work needs none.
                    scratch = tmp_pool.tile([P, 16], F32, tag="scratch", name="scratch", bufs=4)
                    sb = 3 * (c % 4)
                    nc.vector.memset(pos_tile[:, 0:1], 0.0)
                    nc.vector.tensor_copy(scratch[:, sb : sb + 1], g_tile[:, 0:1])
                    nc.vector.tensor_copy(scratch[:, sb + 1 : sb + 2], pin_tile[:, 0:1])
                    nc.vector.tensor_copy(scratch[:, sb + 2 : sb + 3], o_tile[:, 0:1])

                    # pos[0] = root_pos
                    nc.vector.tensor_copy(
                        pos3[:, :, 0, :],
                        pin_tile.rearrange("p (n i) -> p n i", n=nf),
                    )

                    prods = [
                        tmp_pool.tile([P, nf * 27], F32, tag=f"prod{k}", name=f"prod{k}")
                        for k in range(3)
                    ]
                    # Per-G packed views so the used region is fully
                    # contiguous (needed for dense DMA-accumulate adds).
                    pvG = {
                        G_: [
                            t[:, : nf * 9 * G_].rearrange(
                                "p (n g f) -> p n g f", n=nf, g=G_, f=9
                            )
                            for t in prods
                        ]
                        for G_ in (2, 3)
                    }
                    pfG = {
                        G_: [
                            t[:, : nf * 9 * G_].rearrange(
                                "p (n gf) -> p n gf", n=nf
                            )
                            for t in prods
                        ]
                        for G_ in (2, 3)
                    }
                    # ptmp[n, g, k, i] = off[j0+g, k] * G[parent(j0+g)][:, i, k]
                    ptmps = [
                        tmp_pool.tile([P, nf * 27], F32, tag=f"ptmp{x}", name=f"ptmp{x}")
                        .rearrange("p (n g k i) -> p n g k i", n=nf, g=3, k=3)
                        for x in range(2)
                    ]

                    def emit_pos_adds(gi):
                        j0, G, p0, pstep = GROUPS[gi]
                        ptmp = ptmps[gi % 2]
                        if pstep == 0:
                            ppos = pos3[:, :, p0 : p0 + 1, :].broadcast_to(
                                [P, nf, G, 3]
                            )
                        else:
                            ppos = pos3[:, :, p0 : p0 + G, :]
                        pae = _eng(nc, pos_add_engine)
                        s01 = ptmp[:, :, :G, 0, :]
                        pae.tensor_add(s01, s01, ptmp[:, :, :G, 1, :])
                        pae.tensor_add(s01, s01, ptmp[:, :, :G, 2, :])
                        pae.tensor_add(pos3[:, :, j0 : j0 + G, :], s01, ppos)

                    for gi, (j0, G, p0, pstep) in enumerate(GROUPS if do_compute else []):
                        tail = False
                        # rotations: prod_k[n,g,i,:] = G[parent](i,k) * L[child](k,:)
                        for k in range(3):
                            eng = _eng(nc, "gpsimd" if tail else mul_k_engines[k])
                            for i in range(3):
                                if pstep == 0:
                                    par = g5[:, :, p0 : p0 + 1, i, k]
                                else:
                                    par = g5[:, :, p0 : p0 + G, i, k]
                                par = par.unsqueeze(-1).broadcast_to([P, nf, G, 3])
                                chi = g5[:, :, j0 : j0 + G, k, :]
                                out = pvG[G][k][:, :, :, i * 3 : (i + 1) * 3]
                                # broadcast operand on in1 (rd1): a 0-step
                                # AP on in0 costs ~30% extra; on in1 it's free
                                eng.tensor_mul(out, chi, par)
                        a0 = pfG[G][0]
                        a1 = pfG[G][1]
                        a2 = pfG[G][2]
                        if add_engines[0] == "dma":
                            # prod0 += prod1 in the SDMA CCE (dense
                            # SBUF->SBUF accumulate) — frees DVE cycles
                            nc.gpsimd.dma_start(
                                prods[0][:, : nf * 9 * G],
                                prods[1][:, : nf * 9 * G],
                                accum_op=add,
                            )
                        else:
                            ea = "gpsimd" if tail else add_engines[0]
                            _eng(nc, ea).tensor_add(a0, a0, a1)
                        eb = "gpsimd" if tail else add_engines[1]
                        _eng(nc, eb).tensor_add(
                            g3[:, :, j0 * 9 : (j0 + G) * 9], a0, a2
                        )

                        # positions: p[j] = p[par] + sum_k G[par][:,k] * off[j,k]
                        if pos_mode == "stt_dve":
                            for j in range(j0, j0 + G):
                                par_j = PARENTS[j]
                                dst = pos3[:, :, j, :]
                                src = pos3[:, :, par_j, :]
                                for k in range(3):
                                    nc.vector.scalar_tensor_tensor(
                                        dst, g5[:, :, par_j, :, k], o_col(j, k), src,
                                        mult, add,
                                    )
                                    src = dst
                        else:  # act_pool: muls on ScalarE, grouped adds on GPSIMD,
                            # pos-adds deferred one group so pool's add1 of the
                            # next group isn't queued behind ACT-fed pos work
                            ptmp = ptmps[gi % 2]
                            for j in range(j0, j0 + G):
                                par_j = PARENTS[j]
                                for k in range(3):
                                    nc.scalar.mul(
                                        ptmp[:, :, j - j0, k, :],
                                        g5[:, :, par_j, :, k],
                                        o_col(j, k),
                                    )
                            emit_pos_adds(gi)

                    if c + 1 < nchunk:
                        stage_in(c + 1)
                    if do_dma:
                        nc.sync.dma_start(go_v[c], g_tile[:])
                        nc.sync.dma_start(po_v[c], pos_tile[:])
                    else:
                        nc.sync.dma_start(go_v[c][:, 0:8], g_tile[:, 0:8])
                        nc.sync.dma_start(po_v[c][:, 0:8], pos_tile[:, 0:8])
    if split_waits:
        _split_waits(nc)
    return nc


def _run(local_rots, root_pos, offsets, trace=False, nf=NF, repeat=1, **bkw):
    global LAST_RESULT
    lr = np.ascontiguousarray(np.asarray(local_rots, np.float32)).reshape(B, 216)
    rp = np.ascontiguousarray(np.asarray(root_pos, np.float32)).reshape(B, 3)
    ob = np.ascontiguousarray(
        np.tile(np.asarray(offsets, np.float32).reshape(1, 72), (P, 1))
    )
    nc = build(BCORE, nf, repeat=repeat, **bkw)
    in_maps = [
        {
            "local_rots": lr[i * BCORE : (i + 1) * BCORE],
            "root_pos": rp[i * BCORE : (i + 1) * BCORE],
            "offsets_bcast": ob,
        }
        for i in range(NCORES)
    ]
    res = run_bass_kernel_spmd(nc, in_maps, list(range(NCORES)), trace=trace)
    LAST_RESULT = res
    pos = np.concatenate([res.results[i]["gpos"] for i in range(NCORES)], axis=0)
    rots = np.concatenate([res.results[i]["grots"] for i in range(NCORES)], axis=0)
    return (
        pos.reshape(B, NUM_JOINTS, 3).astype(np.float32),
        rots.reshape(B, NUM_JOINTS, 3, 3).astype(np.float32),
    )


def kernel(local_rots, root_pos, offsets):
    return _run(local_rots, root_pos, offsets, trace=False)


# revision 41
# speedup vs baseline: 1.1049x; 1.1049x over previous
"""Forward-kinematics (SMPL 24-joint) Bass kernel for 8 trn2 NeuronCores.

Layout: batch sharded 8 ways (data parallel). Per core, chunks of
[128 partitions x NF] batch elements; each partition row holds NF
elements' data contiguously (element-major: 216 floats per element for
rotations), so every DMA is a dense 2D copy.

Compute: joints processed in 9 tree-level groups with uniform child /
parent index strides, so one multi-dim-AP instruction covers a whole
group. Rotation update G[j] = G[parent] @ L[j] is done as 9 broadcast
muls (one per (i,k)) into per-k product tiles + 2 adds, writing G[j]
in-place over L[j] in the input tile. Position update
p[j] = p[parent] + G[parent] @ offsets[j] runs off the critical path:
per-joint muls on ScalarE (offsets replicated host-side into a [128, 72]
per-partition-scalar tile) and grouped adds on GPSIMD, while the serial
rotation chain stays entirely on VectorE (cross-engine handoffs inside
the chain measured slower every time). Broadcast (0-step) operands are
always passed as in1 — on in0 they cost ~30% extra on the DVE.
"""

import contextlib
import sys

sys.path.insert(0, "/opt/trn_rl_repo")

import numpy as np

import concourse.bass as bass
import concourse.mybir as mybir
import concourse.tile as tile
from concourse.bass_utils import run_bass_kernel_spmd

F32 = mybir.dt.float32

NUM_JOINTS = 24
PARENTS = [-1, 0, 0, 0, 1, 2, 3, 4, 5, 6, 7, 8, 9, 9, 9, 12, 13, 14, 16, 17, 18, 19, 20, 21]
B = 262144
NCORES = 8
BCORE = B // NCORES  # 32768
P = 128

# (j0, group_size, parent0, parent_step) — topological groups where both the
# child joints and their parents are index-arithmetic (parent_step 0 = shared
# parent broadcast across the group).
GROUPS = [
    (1, 3, 0, 0),
    (4, 3, 1, 1),
    (7, 3, 4, 1),
    (10, 3, 7, 1),
    (13, 2, 9, 0),
    (15, 3, 12, 1),
    (18, 2, 16, 1),
    (20, 2, 18, 1),
    (22, 2, 20, 1),
]

# Engine assignment knobs (flip for load-balancing experiments).
MUL_K_ENGINES = ["vector", "vector", "vector"]  # engine for k=0,1,2 rot muls
ADD_ENGINES = ["vector", "vector"]              # engines for the two rot adds
POS_MODE = "act_pool"      # "stt_dve" (fused STT chains on DVE) or "act_pool"
                           # (muls on ScalarE, grouped adds on GPSIMD)
NF = 64                    # free-dim batch elements per chunk

LAST_RESULT = None         # test.py reads exec_time_ns off this


def _eng(nc, name):
    return {"vector": nc.vector, "gpsimd": nc.gpsimd, "scalar": nc.scalar}[name]


_TPB_ENGINES = ("EngineType.DVE", "EngineType.Pool", "EngineType.Activation",
                "EngineType.PE")


def _split_waits(nc):
    """Walrus can embed only ONE sync-wait per TPB compute instruction and
    refuses to compile more. Hoist extra waits onto same-engine NoOps inserted
    immediately before the overloaded instruction (identical semantics: the
    engine stream stalls at the same point)."""
    uid = 0
    for blk in nc.m.functions[0].blocks:
        il = blk.instructions
        out, changed = [], False
        for ins in il:
            si = ins.sync_info
            waits = (si.on_wait if si else None) or []
            if len(waits) > 1:
                for w in waits[:-1]:
                    nop = mybir.InstNoOp(name=f"waitnop-{uid}")
                    uid += 1
                    nop.engine = ins.engine
                    nop.sync_info = mybir.SyncInfo(on_wait=[w], on_update=[])
                    out.append(nop)
                si.on_wait = [waits[-1]]
                changed = True
            out.append(ins)
        if changed:
            blk.instructions = out


def build(bcore=BCORE, nf=NF, mul_k_engines=None, add_engines=None, pos_mode=None,
          repeat=1, split_waits=True, do_dma=True, do_compute=True, io_bufs=2,
          pos_add_engine="gpsimd"):
    mul_k_engines = mul_k_engines or MUL_K_ENGINES
    add_engines = add_engines or ADD_ENGINES
    pos_mode = pos_mode or POS_MODE
    nchunk = bcore // (P * nf)
    assert nchunk * P * nf == bcore

    nc = bass.Bass()
    lr = nc.dram_tensor("local_rots", [bcore, 216], F32, kind="ExternalInput")
    rp = nc.dram_tensor("root_pos", [bcore, 3], F32, kind="ExternalInput")
    ob = nc.dram_tensor("offsets_bcast", [P, 72], F32, kind="ExternalInput")
    g_out = nc.dram_tensor("grots", [bcore, 216], F32, kind="ExternalOutput")
    p_out = nc.dram_tensor("gpos", [bcore, 72], F32, kind="ExternalOutput")

    lr_v = lr[:].rearrange("(c p n) e -> c p (n e)", p=P, n=nf)
    rp_v = rp[:].rearrange("(c p n) e -> c p (n e)", p=P, n=nf)
    go_v = g_out[:].rearrange("(c p n) e -> c p (n e)", p=P, n=nf)
    po_v = p_out[:].rearrange("(c p n) e -> c p (n e)", p=P, n=nf)

    mult = mybir.AluOpType.mult
    add = mybir.AluOpType.add

    with tile.TileContext(nc) as tc:
        with (
            tc.tile_pool(name="io", bufs=io_bufs) as io_pool,
            tc.tile_pool(name="tmp", bufs=1) as tmp_pool,
            tc.tile_pool(name="const", bufs=1) as const_pool,
        ):
            o_tile = const_pool.tile([P, 72], F32)
            nc.sync.dma_start(o_tile[:], ob[:])
            o_col = lambda j, k: o_tile[:, j * 3 + k : j * 3 + k + 1]

            rep_ctx = (
                tc.For_i(
                    0, repeat, 1,
                    hint_engines=(
                        mybir.EngineType.DVE,
                        mybir.EngineType.Pool,
                        mybir.EngineType.Activation,
                        mybir.EngineType.SP,
                    ),
                )
                if repeat > 1
                else contextlib.nullcontext()
            )
            with rep_ctx:
                tiles = {}

                def stage_in(c):
                    # Write-absorber: claim the recycled SBUF slot with a tiny
                    # DVE write BEFORE the DMA fills it, so the slot's
                    # write-after-read waits (vs the chunk-2-ago output DMA)
                    # land here instead of on a real compute instruction.
                    g_tile = io_pool.tile([P, nf * 216], F32, tag="io_rots",
                                          name="g_tile")
                    pin_tile = io_pool.tile([P, nf * 3], F32, tag="io_rootpos",
                                            name="pin_tile")
                    pos_tile = io_pool.tile([P, nf * 72], F32, tag="io_pos",
                                            name="pos_tile")
                    nc.vector.memset(g_tile[:, 0:1], 0.0)
                    if do_dma:
                        nc.sync.dma_start(g_tile[:], lr_v[c])
                        nc.sync.dma_start(pin_tile[:], rp_v[c])
                    else:
                        nc.vector.memset(pin_tile[:, 0:1], 0.0)
                    tiles[c] = (g_tile, pin_tile, pos_tile)

                stage_in(0)
                for c in range(nchunk):
                    g_tile, pin_tile, pos_tile = tiles.pop(c)

                    # 5-D view of the rotation data: [p, n, j, i, k]
                    g5 = g_tile.rearrange("p (n j i k) -> p n j i k", n=nf, j=24, i=3, k=3)
                    g3 = g_tile.rearrange("p (n e) -> p n e", n=nf)
                    pos3 = pos_tile.rearrange("p (n j i) -> p n j i", n=nf, j=24)

                    # Wait-absorbers: TPB compute instructions can embed only
                    # ONE sync wait; walrus refuses to compile an instruction
                    # that needs more. Touch every freshly-DMA'd / released
                    # region with tiny same-engine ops first so each carries
                    # exactly one wait and the real work needs none.
                    scratch = tmp_pool.tile([P, 32], F32, tag="scratch", name="scratch", bufs=4)
                    sb = 6 * (c % 4)
                    nc.scalar.copy(scratch[:, sb : sb + 1], g_tile[:, 0:1])
                    nc.vector.tensor_copy(scratch[:, sb + 3 : sb + 4], g_tile[:, 1:2])
                    nc.scalar.copy(scratch[:, sb + 1 : sb + 2], pin_tile[:, 0:1])
                    nc.scalar.copy(scratch[:, sb + 2 : sb + 3], o_tile[:, 0:1])

                    # pos[0] = root_pos (ACT: keeps DVE free for the rot chain)
                    nc.scalar.copy(
                        pos3[:, :, 0, :],
                        pin_tile.rearrange("p (n i) -> p n i", n=nf),
                    )

                    prods = [
                        tmp_pool.tile([P, nf * 27], F32, tag=f"prod{k}", name=f"prod{k}")
                        for k in range(3)
                    ]
                    # Per-G packed views so the used region is fully
                    # contiguous (needed for dense DMA-accumulate adds).
                    pvG = {
                        G_: [
                            t[:, : nf * 9 * G_].rearrange(
                                "p (n g f) -> p n g f", n=nf, g=G_, f=9
                            )
                            for t in prods
                        ]
                        for G_ in (2, 3)
                    }
                    pfG = {
                        G_: [
                            t[:, : nf * 9 * G_].rearrange(
                                "p (n gf) -> p n gf", n=nf
                            )
                            for t in prods
                        ]
                        for G_ in (2, 3)
                    }
                    # ptmp[n, g, k, i] = off[j0+g, k] * G[parent(j0+g)][:, i, k]
                    ptmps = [
                        tmp_pool.tile([P, nf * 27], F32, tag=f"ptmp{x}", name=f"ptmp{x}")
                        .rearrange("p (n g k i) -> p n g k i", n=nf, g=3, k=3)
                        for x in range(2)
                    ]

                    def emit_pos_adds(gi):
                        j0, G, p0, pstep = GROUPS[gi]
                        ptmp = ptmps[gi % 2]
                        if pstep == 0:
                            ppos = pos3[:, :, p0 : p0 + 1, :].broadcast_to(
                                [P, nf, G, 3]
                            )
                        else:
                            ppos = pos3[:, :, p0 : p0 + G, :]
                        pae = _eng(nc, pos_add_engine)
                        s01 = ptmp[:, :, :G, 0, :]
                        pae.tensor_add(s01, s01, ptmp[:, :, :G, 1, :])
                        pae.tensor_add(s01, s01, ptmp[:, :, :G, 2, :])
                        pae.tensor_add(pos3[:, :, j0 : j0 + G, :], s01, ppos)

                    for gi, (j0, G, p0, pstep) in enumerate(GROUPS if do_compute else []):
                        tail = False
                        # rotations: prod_k[n,g,i,:] = G[parent](i,k) * L[child](k,:)
                        for k in range(3):
                            eng = _eng(nc, "gpsimd" if tail else mul_k_engines[k])
                            for i in range(3):
                                if pstep == 0:
                                    par = g5[:, :, p0 : p0 + 1, i, k]
                                else:
                                    par = g5[:, :, p0 : p0 + G, i, k]
                                par = par.unsqueeze(-1).broadcast_to([P, nf, G, 3])
                                chi = g5[:, :, j0 : j0 + G, k, :]
                                out = pvG[G][k][:, :, :, i * 3 : (i + 1) * 3]
                                # broadcast operand on in1 (rd1): a 0-step
                                # AP on in0 costs ~30% extra; on in1 it's free
                                eng.tensor_mul(out, chi, par)
                        a0 = pfG[G][0]
                        a1 = pfG[G][1]
                        a2 = pfG[G][2]
                        if add_engines[0] == "dma":
                            # prod0 += prod1 in the SDMA CCE (dense
                            # SBUF->SBUF accumulate) — frees DVE cycles
                            nc.gpsimd.dma_start(
                                prods[0][:, : nf * 9 * G],
                                prods[1][:, : nf * 9 * G],
                                accum_op=add,
                            )
                        else:
                            ea = "gpsimd" if tail else add_engines[0]
                            _eng(nc, ea).tensor_add(a0, a0, a1)
                        eb = "gpsimd" if tail else add_engines[1]
                        _eng(nc, eb).tensor_add(
                            g3[:, :, j0 * 9 : (j0 + G) * 9], a0, a2
                        )

                        # positions: p[j] = p[par] + sum_k G[par][:,k] * off[j,k]
                        if pos_mode == "stt_dve":
                            for j in range(j0, j0 + G):
                                par_j = PARENTS[j]
                                dst = pos3[:, :, j, :]
                                src = pos3[:, :, par_j, :]
                                for k in range(3):
                                    nc.vector.scalar_tensor_tensor(
                                        dst, g5[:, :, par_j, :, k], o_col(j, k), src,
                                        mult, add,
                                    )
                                    src = dst
                        else:  # act_pool: muls on ScalarE, grouped adds on GPSIMD,
                            # pos-adds deferred one group so pool's add1 of the
                            # next group isn't queued behind ACT-fed pos work
                            ptmp = ptmps[gi % 2]
                            for j in range(j0, j0 + G):
                                par_j = PARENTS[j]
                                for k in range(3):
                                    nc.scalar.mul(
                                        ptmp[:, :, j - j0, k, :],
                                        g5[:, :, par_j, :, k],
                                        o_col(j, k),
                                    )
                            emit_pos_adds(gi)

                    if c + 1 < nchunk:
                        stage_in(c + 1)
                    if do_dma:
                        nc.sync.dma_start(go_v[c], g_tile[:])
                        nc.sync.dma_start(po_v[c], pos_tile[:])
                    else:
                        nc.sync.dma_start(go_v[c][:, 0:8], g_tile[:, 0:8])
                        nc.sync.dma_start(po_v[c][:, 0:8], pos_tile[:, 0:8])
    if split_waits:
        _split_waits(nc)
    return nc


def _run(local_rots, root_pos, offsets, trace=False, nf=NF, repeat=1, **bkw):
    global LAST_RESULT
    lr = np.ascontiguousarray(np.asarray(local_rots, np.float32)).reshape(B, 216)
    rp = np.ascontiguousarray(np.asarray(root_pos, np.float32)).reshape(B, 3)
    ob = np.ascontiguousarray(
        np.tile(np.asarray(offsets, np.float32).reshape(1, 72), (P, 1))
    )
    nc = build(BCORE, nf, repeat=repeat, **bkw)
    in_maps = [
        {
            "local_rots": lr[i * BCORE : (i + 1) * BCORE],
            "root_pos": rp[i * BCORE : (i + 1) * BCORE],
            "offsets_bcast": ob,
        }
        for i in range(NCORES)
    ]
    res = run_bass_kernel_spmd(nc, in_maps, list(range(NCORES)), trace=trace)
    LAST_RESULT = res
    pos = np.concatenate([res.results[i]["gpos"] for i in range(NCORES)], axis=0)
    rots = np.concatenate([res.results[i]["grots"] for i in range(NCORES)], axis=0)
    return (
        pos.reshape(B, NUM_JOINTS, 3).astype(np.float32),
        rots.reshape(B, NUM_JOINTS, 3, 3).astype(np.float32),
    )


def kernel(local_rots, root_pos, offsets):
    return _run(local_rots, root_pos, offsets, trace=False)
